# revision 20
# baseline (speedup 1.0000x reference)
"""DuoAttention kernel for 8 TRN2 NeuronCores.

Math note: the reference's WINDOW == seq_len, so `local` and `full` are the
same MHA computation. The kernel computes one MHA pass; the duo gate reduces
to a per-batch scalar factor c[i] = (m[i] < 0.1) ? (1 - m[i]) : 1.0 applied
in the broadcast combine out[i, j] = c[i] * mha[j] (shape [B, B, S, D]).

Sharding: data-parallel over batch (2) x tensor-parallel over head groups
(4 groups x 4 heads). Each core computes QKV projections for its 256
features, attention for its 4 heads, and a partial output projection
(contribution of its 256 o-features to all 1024 output dims). The host sums
the 4 partials per batch, adds the output bias, and applies the gate.

Per-core kernel layout (all matmuls bf16, fp32 accumulation):
  - activations arrive host-transposed: qT/kT/vT [1024, 2048] bf16
  - qp/kp stored transposed [256f, 2048s]; scores computed transposed
    [keys, queries] so attn@v contracts over keys on partitions
  - rowsum via an appended ones-column in the attn@v stationary operand
  - softmax skips max-subtraction (logits are bounded ~ +-5 by construction)
"""

import sys

import numpy as np
import ml_dtypes

_REPO = "/opt/trn_rl_repo"
if _REPO not in sys.path:
    sys.path.insert(0, _REPO)

import concourse.bass as bass
import concourse.bacc as bacc
import concourse.mybir as mybir
import concourse.tile as tile
from concourse.bass_utils import run_bass_kernel_spmd

B, S, D, H = 2, 2048, 1024, 16
NCORES = 8
GROUPS = 4            # head groups (tensor parallel)
HPG = H // GROUPS     # 4 heads per group
DH = D // H           # 64
GF = HPG * DH         # 256 features per group
DC = D // 128         # 8 contraction chunks of 128
ST = S // 128         # 16 seq tiles of 128
QT = S // 512         # 4 query tiles of 512
KT = S // 128         # 16 key tiles of 128

BF16 = mybir.dt.bfloat16
F32 = mybir.dt.float32


def build_nc(dbg=False):
    nc = bacc.Bacc("TRN2", target_bir_lowering=False, debug=False,
                   num_devices=NCORES)

    qT = nc.dram_tensor("qT", [D, S], BF16, kind="ExternalInput").ap()
    kT = nc.dram_tensor("kT", [D, S], BF16, kind="ExternalInput").ap()
    vT = nc.dram_tensor("vT", [D, S], BF16, kind="ExternalInput").ap()
    wqT = nc.dram_tensor("wqT", [D, GF], BF16, kind="ExternalInput").ap()
    wkT = nc.dram_tensor("wkT", [D, GF], BF16, kind="ExternalInput").ap()
    wvT = nc.dram_tensor("wvT", [D, GF], BF16, kind="ExternalInput").ap()
    woT = nc.dram_tensor("woT", [GF, D], BF16, kind="ExternalInput").ap()
    bq = nc.dram_tensor("bq", [GF], F32, kind="ExternalInput").ap()
    bk = nc.dram_tensor("bk", [GF], F32, kind="ExternalInput").ap()
    bv = nc.dram_tensor("bv", [GF], BF16, kind="ExternalInput").ap()
    out = nc.dram_tensor("out_part", [S, D], F32, kind="ExternalOutput").ap()

    dbg_t = {}
    if dbg:
        for name, shape, dt in (
            ("dbg_qp", [128, S], BF16), ("dbg_kp", [128, S], BF16),
            ("dbg_vp", [128, HPG * (DH + 1)], BF16),
            ("dbg_sc", [128, 1024], F32), ("dbg_ex", [128, 1024], BF16),
            ("dbg_av", [DH + 1, 512], F32), ("dbg_rc", [1, 512], F32),
            ("dbg_bc", [64, 512], F32), ("dbg_ot", [128, 2, 512], BF16),
        ):
            dbg_t[name] = nc.dram_tensor(name, shape, dt,
                                         kind="ExternalOutput").ap()

    qT3 = qT.rearrange("(c p) s -> p c s", p=128)
    kT3 = kT.rearrange("(c p) s -> p c s", p=128)
    vT3 = vT.rearrange("(c p) s -> p c s", p=128)

    with tile.TileContext(nc) as tc:
        with (
            tc.tile_pool(name="const", bufs=1) as const,
            tc.tile_pool(name="acts", bufs=1) as acts,
            tc.tile_pool(name="sc", bufs=2, space="PSUM") as scp,
            tc.tile_pool(name="misc", bufs=4, space="PSUM") as miscp,
            tc.tile_pool(name="exp", bufs=3) as exps,
            tc.tile_pool(name="ot", bufs=2) as otp,
            tc.tile_pool(name="small", bufs=2) as small,
            tc.tile_pool(name="outs", bufs=2) as outsp,
        ):
            # ---------------- weights / biases ----------------
            wk_sb = const.tile([128, DC, GF], BF16, tag="wk")
            nc.sync.dma_start(out=wk_sb, in_=wkT.rearrange("(c p) f -> p c f", p=128))
            wq_sb = const.tile([128, DC, GF], BF16, tag="wq")
            nc.sync.dma_start(out=wq_sb, in_=wqT.rearrange("(c p) f -> p c f", p=128))
            wv_sb = const.tile([128, DC, GF], BF16, tag="wv")
            nc.sync.dma_start(out=wv_sb, in_=wvT.rearrange("(c p) f -> p c f", p=128))
            wo_sb = const.tile([128, 2, D], BF16, tag="wo")
            nc.sync.dma_start(out=wo_sb, in_=woT.rearrange("(c p) n -> p c n", p=128))

            bk_sb = const.tile([128, 2], F32, tag="bk")
            nc.sync.dma_start(out=bk_sb, in_=bk.rearrange("(t p) -> p t", p=128))
            bq_sb = const.tile([128, 2], F32, tag="bq")
            nc.sync.dma_start(out=bq_sb, in_=bq.rearrange("(t p) -> p t", p=128))
            bv_sb = const.tile([1, GF], BF16, tag="bv")
            nc.sync.dma_start(out=bv_sb, in_=bv.rearrange("(o f) -> o f", o=1))
            ones_sb = const.tile([1, 128], BF16, tag="ones")
            nc.vector.memset(ones_sb, 1.0)

            # ---------------- raw activation loads (per d-chunk) ----------
            k_sb = acts.tile([128, DC, S], BF16, tag="k")
            q_sb = acts.tile([128, DC, S], BF16, tag="q")
            v_sb = acts.tile([128, DC, S], BF16, tag="v")
            for dc in range(DC):
                nc.sync.dma_start(out=k_sb[:, dc, :], in_=kT3[:, dc, :])
            for dc in range(DC):
                nc.sync.dma_start(out=q_sb[:, dc, :], in_=qT3[:, dc, :])
            for dc in range(DC):
                nc.sync.dma_start(out=v_sb[:, dc, :], in_=vT3[:, dc, :])

            # projected tensors: per-pair tiles for fine-grained deps
            kp_t = [acts.tile([128, S], BF16, tag=f"kp{p}", name=f"kp{p}") for p in range(2)]
            qp_t = [acts.tile([128, S], BF16, tag=f"qp{p}", name=f"qp{p}") for p in range(2)]
            vp_t = [acts.tile([128, HPG * (DH + 1)], BF16, tag=f"vp{st}",
                              name=f"vp{st}") for st in range(ST)]

            def proj_fs_group(w_sb, b_sb, dst_t, x_sb, ft, st4):
                """project one [128f, 512s] tile of q or k (pair ft)."""
                ps = miscp.tile([128, 512], F32, tag="misc", name="ps_fs")
                for dc in range(DC):
                    nc.tensor.matmul(
                        ps,
                        w_sb[:, dc, 128 * ft:128 * ft + 128],
                        x_sb[:, dc, 512 * st4:512 * st4 + 512],
                        start=(dc == 0), stop=(dc == DC - 1),
                    )
                nc.vector.tensor_scalar_add(
                    dst_t[ft][:, 512 * st4:512 * st4 + 512],
                    ps, b_sb[:, ft:ft + 1],
                )

            def proj_v_group(st):
                ps = miscp.tile([128, 512], F32, tag="misc", name="ps_v")
                for dc in range(DC):
                    nc.tensor.matmul(
                        ps[:, 0:GF],
                        v_sb[:, dc, 128 * st:128 * st + 128],
                        wv_sb[:, dc, :],
                        start=(dc == 0), stop=False,
                    )
                nc.tensor.matmul(ps[:, 0:GF], ones_sb[0:1, :], bv_sb[0:1, :],
                                 start=False, stop=True)
                vph = vp_t[st].rearrange("p (h c) -> p h c", c=DH + 1)
                nc.vector.memset(vph[:, :, DH:DH + 1], 1.0)
                nc.vector.tensor_copy(
                    vph[:, :, 0:DH],
                    ps[:, 0:GF].rearrange("p (h c) -> p h c", c=DH),
                )

            def outproj_group(qt, oT_prev, sj):
                outt = outsp.tile([128, D], F32, tag="os", name="outt")
                for do in range(2):
                    ps = miscp.tile([128, 512], F32, tag="misc", name="ps_o")
                    for fc in range(2):
                        nc.tensor.matmul(
                            ps,
                            oT_prev[:, fc, 128 * sj:128 * sj + 128],
                            wo_sb[:, fc, 512 * do:512 * do + 512],
                            start=(fc == 0), stop=(fc == 1),
                        )
                    nc.vector.tensor_copy(
                        outt[:, 512 * do:512 * do + 512], ps)
                row = 512 * qt + 128 * sj
                nc.sync.dma_start(out=out[row:row + 128, :], in_=outt)

            # Deferred PE-side jobs drip-fed into the attention loop: the
            # attention inner loop is ACT(exp)-bound, so projection and
            # output-projection groups fill the PE slack without delaying
            # the exp stream.
            # Ordering invariant: Tile derives dependencies from TRACE
            # order, so every producer group must be emitted before its
            # first consumer. vp[st] is consumed at av(kt=st) of (qt0,p0);
            # qp1/kp1 are consumed by (qt0,p1). Draining two jobs per kt
            # slot during (qt0,p0) satisfies both.
            jobs = []
            qkjobs = []
            qkjobs += [lambda s=s: proj_fs_group(wq_sb, bq_sb, qp_t, q_sb, 1, s)
                       for s in range(QT)]
            qkjobs += [lambda s=s: proj_fs_group(wk_sb, bk_sb, kp_t, k_sb, 1, s)
                       for s in range(QT)]

            # prologue: just enough for attention (qt0, pair0) to start;
            # vp[st] is emitted just-in-time inside the (qt0,p0) kt loop
            for s in range(QT):
                proj_fs_group(wk_sb, bk_sb, kp_t, k_sb, 0, s)
            for s in range(QT):
                proj_fs_group(wq_sb, bq_sb, qp_t, q_sb, 0, s)

            if dbg:
                nc.sync.dma_start(out=dbg_t["dbg_qp"], in_=qp_t[0])
                nc.sync.dma_start(out=dbg_t["dbg_kp"], in_=kp_t[0])
                nc.sync.dma_start(out=dbg_t["dbg_vp"], in_=vp_t[0])

            for qt in range(QT):
                oT_t = otp.tile([128, 2, 512], BF16, tag="ot", name="oT_t")
                for p in range(2):
                    av0 = miscp.tile([DH + 1, 512], F32, tag="misc", name="av0")
                    av1 = miscp.tile([DH + 1, 512], F32, tag="misc", name="av1")
                    h0, h1 = 2 * p, 2 * p + 1
                    for kt in range(KT):
                        sc = scp.tile([128, 1024], F32, tag="sc", name="sc")
                        # row-packed pair: head h0 on array rows 0-63,
                        # head h1 on rows 64-127
                        nc.tensor.matmul(
                            sc[:, 0:512],
                            kp_t[p][0:64, 128 * kt:128 * kt + 128],
                            qp_t[p][0:64, 512 * qt:512 * qt + 512],
                            start=True, stop=True,
                        )
                        nc.tensor.matmul(
                            sc[:, 512:1024],
                            kp_t[p][64:128, 128 * kt:128 * kt + 128],
                            qp_t[p][64:128, 512 * qt:512 * qt + 512],
                            start=True, stop=True,
                        )
                        ex = exps.tile([128, 1024], BF16, tag="exp", name="ex")
                        nc.scalar.activation(
                            out=ex, in_=sc,
                            func=mybir.ActivationFunctionType.Exp,
                            scale=1.0 / np.sqrt(DH),
                        )
                        if dbg and qt == 0 and p == 0 and kt == 0:
                            stg = small.tile([128, 1024], F32, tag="dbgsc")
                            nc.vector.tensor_copy(stg, sc)
                            nc.sync.dma_start(out=dbg_t["dbg_sc"], in_=stg)
                            nc.sync.dma_start(out=dbg_t["dbg_ex"], in_=ex)
                        if qt == 0 and p == 0:
                            proj_v_group(kt)
                            if kt >= KT - 8 and qkjobs:
                                qkjobs.pop(0)()
                        elif jobs:
                            jobs.pop(0)()
                        nc.tensor.matmul(
                            av0, vp_t[kt][:, 65 * h0:65 * h0 + 65],
                            ex[:, 0:512],
                            start=(kt == 0), stop=(kt == KT - 1),
                        )
                        nc.tensor.matmul(
                            av1, vp_t[kt][:, 65 * h1:65 * h1 + 65],
                            ex[:, 512:1024],
                            start=(kt == 0), stop=(kt == KT - 1),
                        )
                    for j, av in ((0, av0), (1, av1)):
                        # copy the whole accumulator to sbuf immediately so
                        # the psum slot frees early; rowsum sits on partition
                        # 64 and engines can't move across partitions, so DMA
                        # it down to p0 for the reciprocal
                        avs = small.tile([DH + 1, 512], F32, tag="avs")
                        nc.vector.tensor_copy(avs, av)
                        rcs = small.tile([1, 512], F32, tag="rcs")
                        nc.sync.dma_start(out=rcs, in_=avs[DH:DH + 1, :])
                        rc = small.tile([1, 512], F32, tag="rc")
                        nc.vector.reciprocal_approx_fast(rc, rcs)
                        bc = small.tile([64, 512], F32, tag="bc")
                        nc.gpsimd.partition_broadcast(bc, rc)
                        if dbg and qt == 0 and p == 0 and j == 0:
                            nc.sync.dma_start(out=dbg_t["dbg_av"], in_=avs)
                            nc.sync.dma_start(out=dbg_t["dbg_rc"], in_=rc)
                            nc.sync.dma_start(out=dbg_t["dbg_bc"], in_=bc)
                        nc.vector.tensor_mul(
                            oT_t[64 * j:64 * j + 64, p, :], avs[0:DH, :], bc)

                if dbg and qt == 0:
                    nc.sync.dma_start(out=dbg_t["dbg_ot"], in_=oT_t)

                jobs += [lambda q=qt, o=oT_t, sj=sj:
                         outproj_group(q, o, sj) for sj in range(4)]
            while jobs:
                jobs.pop(0)()

    nc.compile()
    return nc


_CACHE = {}


def _get_nc():
    if "nc" not in _CACHE:
        _CACHE["nc"] = build_nc()
    return _CACHE["nc"]


def _prep_inputs(query, key, value, in_proj_w, in_proj_b, out_proj_w):
    bf16 = ml_dtypes.bfloat16
    wq, wk, wv = (in_proj_w[0:D], in_proj_w[D:2 * D], in_proj_w[2 * D:3 * D])
    bq, bk, bv = (in_proj_b[0:D], in_proj_b[D:2 * D], in_proj_b[2 * D:3 * D])

    qT = [np.ascontiguousarray(query[b].T).astype(bf16) for b in range(B)]
    kT = [np.ascontiguousarray(key[b].T).astype(bf16) for b in range(B)]
    vT = [np.ascontiguousarray(value[b].T).astype(bf16) for b in range(B)]

    in_maps = []
    for b in range(B):
        for g in range(GROUPS):
            fs = slice(GF * g, GF * (g + 1))
            in_maps.append({
                "qT": qT[b], "kT": kT[b], "vT": vT[b],
                "wqT": np.ascontiguousarray(wq[fs].T).astype(bf16),
                "wkT": np.ascontiguousarray(wk[fs].T).astype(bf16),
                "wvT": np.ascontiguousarray(wv[fs].T).astype(bf16),
                "woT": np.ascontiguousarray(out_proj_w[:, fs].T).astype(bf16),
                "bq": np.ascontiguousarray(bq[fs]).astype(np.float32),
                "bk": np.ascontiguousarray(bk[fs]).astype(np.float32),
                "bv": np.ascontiguousarray(bv[fs]).astype(bf16),
            })
    return in_maps


def kernel(query, key, value, in_proj_w, in_proj_b, out_proj_w, out_proj_b,
           mask_w, mask_b, _run_kwargs=None):
    query = np.asarray(query, np.float32)
    key = np.asarray(key, np.float32)
    value = np.asarray(value, np.float32)
    in_proj_w = np.asarray(in_proj_w, np.float32)
    in_proj_b = np.asarray(in_proj_b, np.float32)
    out_proj_w = np.asarray(out_proj_w, np.float32)
    out_proj_b = np.asarray(out_proj_b, np.float32)
    mask_w = np.asarray(mask_w, np.float32)
    mask_b = np.asarray(mask_b, np.float32)

    in_maps = _prep_inputs(query, key, value, in_proj_w, in_proj_b, out_proj_w)
    nc = _get_nc()
    res = run_bass_kernel_spmd(nc, in_maps, core_ids=list(range(NCORES)),
                               **(_run_kwargs or {}))
    parts = [r["out_part"] for r in res.results]
    mha = np.stack(
        [sum(parts[b * GROUPS + g] for g in range(GROUPS)) for b in range(B)],
        axis=0,
    ) + out_proj_b[None, None, :].astype(np.float32)

    logit = (query[:, -1] @ mask_w.T + mask_b).astype(np.float64)
    m = (1.0 / (1.0 + np.exp(-logit))).astype(np.float32).reshape(B)
    c = np.where(m < 0.1, np.float32(1.0) - m, np.float32(1.0))

    out_full = c[:, None, None, None] * mha[None, :, :, :]
    if _run_kwargs is not None:
        _CACHE["last_results"] = res
    return out_full.astype(np.float32)
